# revision 41
# baseline (speedup 1.0000x reference)
"""MoCo hard-example-mining loss (topk_masking) on 8 Trainium2 NeuronCores.

Strategy (sharding_hint: shard queue along K):
  The reference computes dist = euclid(feat_q, queue_eff.T) [N=512, K=65536],
  then masked max (hard positive) / min (hard negative) per row, then a
  scalar soft-margin loss.  After the enqueue step, queue_eff columns are:
    - cols [0, 512):  feat_k.T with labels = targets   (the "special" block)
    - cols [512, 64K): original L2-normalized queue columns, labels = 0
  For the zero-label region the mask is row-constant and ||z_j||^2 == 1, so
  per row only an extreme of p_ij = <feat_q_i, z_j> over that region is
  needed — and only ONE side per row:
    - rows with target != 0: the region holds only negatives -> need max_j p
    - rows with target == 0: the region holds only positives -> need min_j p
  Sign-flipping the target==0 rows of feat_q on the host turns both cases
  into a single per-row MAX on device (min_j p = -max_j <-q, z>).  The
  512-column special block and the final scalar loss are computed exactly on
  the host in float64 (trivial cost).

  Device (per core, fp8e4 + DoubleRow): the 65024 zero-label columns are
  padded to 65536 with duplicate columns (harmless for max) and sharded 8192
  per core.  128 DoubleRow matmuls (256-deep contraction, 512-wide moving)
  fill [128, 1024] 2-bank PSUM pairs; one DVE max tensor_reduce per pair
  accumulates into a [128, 4, 8] slot tile; a final tiny reduce + 2 KB DMA
  returns [128, 4] row maxima (row index = m*128 + p).  Host reduces across
  cores.  fp8e4 noise on p (~0.05 abs on extremes ~100) is far inside the
  2e-2 loss tolerance.
"""

import sys
import types
import numpy as np
import ml_dtypes

N, DIM, K, B = 512, 512, 65536, 512
NCORES = 8
KZ = K - B            # zero-label columns
CPC = K // NCORES     # padded columns per core (8192)
NT = CPC // 512       # 512-wide column tiles per core (16)
BIG = 9999999.0

LAST_RESULTS = None   # BassKernelResults of the most recent device run
_NC_CACHE = {}


def _install_axon_hooks_shim():
    """antenv.axon_hooks is absent on this image; bass_utils imports it when
    NTFF tracing is requested.  Provide the tiny get/set module and register
    the ctypes-based NTFF hook so trace=True / BASS_TRACE=1 works."""
    try:
        import antenv  # noqa: F401
    except ImportError:
        return
    if "antenv.axon_hooks" in sys.modules:
        return
    mod = types.ModuleType("antenv.axon_hooks")
    mod._hook = None

    def set_axon_ntff_profile_hook(h):
        mod._hook = h

    def get_axon_ntff_profile_hook():
        return mod._hook

    mod.set_axon_ntff_profile_hook = set_axon_ntff_profile_hook
    mod.get_axon_ntff_profile_hook = get_axon_ntff_profile_hook
    sys.modules["antenv.axon_hooks"] = mod
    sys.modules["antenv"].axon_hooks = mod
    try:
        from trn_agent_boot.trn_boot import _ntff_profile_via_ctypes

        mod._hook = _ntff_profile_via_ctypes("/opt/axon/libaxon_pjrt.so")
    except Exception:
        pass


def _build_nc():
    """Build + compile the per-core Bass program (identical on all cores)."""
    import concourse.bacc as bacc
    import concourse.mybir as mybir
    from concourse.tile import TileContext

    f8 = mybir.dt.float8e4
    f32 = mybir.dt.float32
    DR = mybir.MatmulPerfMode.DoubleRow

    nc = bacc.Bacc("TRN2", debug=False, target_bir_lowering=False)
    bf16 = mybir.dt.bfloat16
    # qT8[p, kk, i] = feat_q'[i, kk*128+p]  (sign-flipped rows for target==0)
    qT = nc.dram_tensor("qT8", [128, 4, N], f8, kind="ExternalInput")
    # slab8[p, 4*n+kk, c] = z[kk*128+p, n*512+c]  (per-core column slab)
    slab = nc.dram_tensor("slab8", [128, 4 * NT, 512], f8, kind="ExternalInput")
    # Running-max accumulators per m block, shipped raw; the final fold
    # happens on the host.  Row index = m*128 + p.
    #   oF: direct DVE-from-PSUM path      oD: ACT-copy + batched DVE TT path
    #   oX: solo half-reduces of the very last PSUM pair (m=3 only)
    oF = nc.dram_tensor("oF", [128, 4, 1024], bf16, kind="ExternalOutput")
    oD = nc.dram_tensor("oD", [128, 4, 2048], bf16, kind="ExternalOutput")
    oX = nc.dram_tensor("oX", [128, 2], bf16, kind="ExternalOutput")

    # Per-m evacuation path of each of the 8 column-pair PSUM tiles:
    #   F = DVE tensor_tensor max straight from PSUM (fp32 src, 1x, ~1.2us)
    #   D = ScalarE copy psum->bf16 (~1.15us) + DVE bf16 TT max (2x, ~0.7us)
    # 2F/6D balances DVE ~25us vs ACT ~27.5us, both under the ~29us matmul
    # stream.  tensor_reduce is NOT used in the hot path (only a 1x uop on
    # TRN2), and GpSimd tensor ops fail the PJRT-side walrus engine check.
    # For the last m the F pairs go last so the D accumulator (and its DMA)
    # retires before the stream ends.
    # Per-m path lists.  D pairs are copied by ScalarE into halves of a
    # shared [128,2048] bt tile; each completed bt folds into the [128,2048]
    # accD with ONE DVE bf16 TT at 2x (~0.6us/pair), leaving DVE ~24.6us and
    # ACT ~22.9us — both clearly under the 27.6us matmul stream.  m3's final
    # pair is two solo half-reduces (path X) so only ~1.4us of DVE + a 1KB
    # DMA trail the final matmul.
    PATHS_M = [
        ["F", "D", "D", "D", "F", "D", "D", "D"],
        ["F", "D", "D", "D", "F", "D", "D", "F"],
        ["F", "D", "D", "D", "F", "D", "D", "F"],
        ["D", "D", "D", "F", "D", "F", "F", "X"],
    ]
    # (m, pair) issue order: 4-way round-robin over the m blocks.  Each slab
    # tile pair is consumed over 4 consecutive steps (~3.6us), far behind the
    # ~1.5us/tile DMA arrival rate, there are no sharp m-boundaries (whose
    # evac-op clusters stalled PSUM slot reuse), and the last m's ScalarE
    # copies spread across the whole stream instead of bunching at the end.
    SCHED = [(m, p) for p in range(NT // 2) for m in range(4)]

    with TileContext(nc) as tc:
        with (
            tc.tile_pool(name="qpool", bufs=1) as qpool,
            tc.tile_pool(name="spool", bufs=16) as spool,
            tc.tile_pool(name="btpool", bufs=8) as btpool,
            tc.tile_pool(name="apool", bufs=4) as apool,
            tc.tile_pool(name="pspool", bufs=4, space="PSUM") as pspool,
        ):
            # HAM warmup: FD-512 matmuls bridge the whole preamble+DMA-latency
            # window (~3.4 us at the cold 1.2 GHz clock) so the PE activity
            # monitor unthrottles to 2.4 GHz right as the real stream starts.
            warm = qpool.tile([128, 512], f8, name="warm")
            nc.gpsimd.memset(warm, 0.0)
            # qt rides the second HWDGE ring (ScalarE) so its ~3us transfer
            # overlaps st0's on the Sync ring and every slab tile lands one
            # issue-slot earlier
            qt = qpool.tile([128, 4, N], f8, name="qt")
            nc.scalar.dma_start(out=qt, in_=qT.ap())
            # dummy ACTIVATE (after the qt issue) so walrus's lazily-inserted
            # ACT_TABLE_LOAD (~2.7 us) runs during the preamble window instead
            # of delaying the first real psum copy
            wact = qpool.tile([128, 8], bf16, name="wact")
            nc.scalar.copy(wact, warm[:, 0:8])
            wps = pspool.tile([128, 2, 512], f32, name="wps", tag="ps")
            for _ in range(9):
                nc.tensor.matmul(wps[0:16, 0, :], warm[:, 0:16], warm)

            # stage the whole 4 MB slab (resident: 16 x 2KB/partition)
            sts = []
            for n in range(NT):
                st = spool.tile([128, 4, 512], f8, name="st", tag="st")
                nc.sync.dma_start(out=st, in_=slab.ap()[:, 4 * n : 4 * n + 4, :])
                sts.append(st)

            paths_by_m = PATHS_M
            accs = {}  # m -> {path: [tile, has_data]}
            pend = {}  # m -> bt tile whose half 0 holds an unfolded D copy
            ACC_W = {"F": 1024, "D": 2048, "X": 2}
            for m, pair in SCHED:
                paths = paths_by_m[m]
                if m not in accs:
                    accs[m] = {}
                    pend[m] = None
                    for pth in set(paths):
                        t = apool.tile(
                            [128, ACC_W[pth]], bf16, name="acc", tag="a" + pth
                        )
                        accs[m][pth] = [t, False]
                ps = pspool.tile([128, 2, 512], f32, name="ps", tag="ps")
                for half in range(2):
                    for kp in range(2):
                        nc.tensor.matmul(
                            ps[:, half, :],
                            qt[:, 2 * kp : 2 * kp + 2, m * 128 : (m + 1) * 128],
                            sts[pair * 2 + half][:, 2 * kp : 2 * kp + 2, :],
                            start=(kp == 0),
                            stop=(kp == 1),
                            perf_mode=DR,
                        )
                psf = ps.rearrange("p a b -> p (a b)")
                pth = paths[pair]
                at, has = accs[m][pth]
                if pth == "X":
                    # two half reduces (tensor_reduce is 1x but only 512 elems
                    # here; the first overlaps the final accumulation group)
                    # and a 1KB trailing DMA
                    nc.vector.tensor_reduce(
                        at[:, 0:1], ps[:, 0, :],
                        axis=mybir.AxisListType.X, op=mybir.AluOpType.max,
                    )
                    nc.vector.tensor_reduce(
                        at[:, 1:2], ps[:, 1, :],
                        axis=mybir.AxisListType.X, op=mybir.AluOpType.max,
                    )
                    nc.sync.dma_start(out=oX.ap(), in_=at)
                    continue
                elif pth == "F":
                    if not has:
                        nc.vector.tensor_copy(at, psf)
                    else:
                        nc.vector.tensor_tensor(at, at, psf, op=mybir.AluOpType.max)
                else:
                    if pend[m] is None:
                        bt = btpool.tile([128, 2048], bf16, name="bt", tag="bt")
                        nc.scalar.copy(bt[:, 0:1024], psf)
                        if paths[pair + 1 :].count("D"):
                            pend[m] = bt
                            continue  # folds (and possibly ships) later
                        # lone trailing D of an odd count: fold the half now
                        # (odd counts always have an earlier fold, so has=True)
                        nc.vector.tensor_tensor(
                            at[:, 0:1024], at[:, 0:1024], bt[:, 0:1024],
                            op=mybir.AluOpType.max,
                        )
                    else:
                        bt = pend[m]
                        pend[m] = None
                        nc.scalar.copy(bt[:, 1024:2048], psf)
                        if not has:
                            nc.vector.tensor_copy(at, bt)
                        else:
                            nc.vector.tensor_tensor(
                                at, at, bt, op=mybir.AluOpType.max
                            )
                accs[m][pth][1] = True
                if paths[pair + 1 :].count(pth) == 0:
                    # last contribution for this accumulator: ship it
                    od = oF if pth == "F" else oD
                    nc.sync.dma_start(out=od.ap()[:, m, :], in_=at)

    nc.compile()
    return nc


def _get_nc():
    if "nc" not in _NC_CACHE:
        _install_axon_hooks_shim()
        _NC_CACHE["nc"] = _build_nc()
    return _NC_CACHE["nc"]


def _host_reference(feat_q, feat_k, targets, queue, queue_label):
    """Exact numpy fallback (float64) — used only if input assumptions
    (zero labels / normalized columns outside the enqueue block) fail."""
    fq = feat_q.astype(np.float64)
    fk = feat_k.astype(np.float64)
    t = targets.astype(np.int64)
    q = queue.astype(np.float64).copy()
    ql = queue_label.astype(np.int64).copy()
    q[:, : fk.shape[0]] = fk.T
    ql[: fk.shape[0]] = t
    xx = (fq * fq).sum(1)[:, None]
    yy = (q * q).sum(0)[None, :]
    sq = xx + yy - 2.0 * (fq @ q)
    dist = np.sqrt(np.clip(sq, 1e-12, None))
    is_pos = t[:, None] == ql[None, :]
    dist_ap = np.max(dist - BIG * (~is_pos), axis=1)
    dist_an = np.min(dist + BIG * is_pos, axis=1)
    return _loss(dist_ap, dist_an)


def _loss(dist_ap, dist_an):
    diff = dist_an - dist_ap
    loss_soft = np.mean(np.logaddexp(0.0, -diff))
    if np.isinf(loss_soft):
        return np.float32(np.mean(np.maximum(dist_ap - dist_an + 0.3, 0.0)))
    return np.float32(loss_soft)


def kernel(feat_q, feat_k, targets, queue, queue_label):
    feat_q = np.asarray(feat_q, dtype=np.float32)
    feat_k = np.asarray(feat_k, dtype=np.float32)
    targets = np.asarray(targets)
    queue = np.asarray(queue, dtype=np.float32)
    queue_label = np.asarray(queue_label)

    t = targets.astype(np.int64)
    Z = queue[:, B:]  # zero-label region, untouched by the enqueue

    # Guards for the two structural assumptions this split relies on.
    ok = not np.any(queue_label != 0)
    if ok:
        sample = np.linspace(0, KZ - 1, 512, dtype=np.int64)
        yy_s = np.einsum("ij,ij->j", Z[:, sample], Z[:, sample], dtype=np.float64)
        ok = bool(np.max(np.abs(yy_s - 1.0)) < 1e-3)
    if not ok:
        return _host_reference(feat_q, feat_k, targets, queue, queue_label)

    # ---- device part: per-row max of feat_q' @ Z over the zero-label region
    fp8 = ml_dtypes.float8_e4m3
    sign = np.where(t == 0, -1.0, 1.0).astype(np.float32)
    fq8 = (feat_q * sign[:, None]).astype(fp8)          # [N, dim]
    qtd = np.ascontiguousarray(fq8.T.reshape(4, 128, N).transpose(1, 0, 2))
    Z8 = Z.astype(fp8)                                   # [dim, KZ]
    in_maps = []
    for c in range(NCORES):
        lo = c * CPC
        hi = min((c + 1) * CPC, KZ)
        sl = np.empty((DIM, CPC), dtype=fp8)
        sl[:, : hi - lo] = Z8[:, lo:hi]
        if hi - lo < CPC:  # pad the tail core with duplicate columns
            sl[:, hi - lo :] = Z8[:, : CPC - (hi - lo)]
        sld = np.ascontiguousarray(
            sl.reshape(4, 128, NT, 512).transpose(1, 2, 0, 3).reshape(128, 4 * NT, 512)
        )
        in_maps.append({"qT8": qtd, "slab8": sld})

    from concourse import bass_utils

    nc = _get_nc()
    res = bass_utils.run_bass_kernel_spmd(nc, in_maps, core_ids=list(range(NCORES)))
    global LAST_RESULTS
    LAST_RESULTS = res

    pmx = np.full(N, -np.inf)
    for c in range(NCORES):
        rc = res.results[c]
        vF = np.asarray(rc["oF"], dtype=np.float64).max(axis=-1)  # [128, 4]
        vD = np.asarray(rc["oD"], dtype=np.float64).max(axis=-1)  # [128, 4]
        v = np.maximum(vF, vD)
        v[:, 3] = np.maximum(
            v[:, 3], np.asarray(rc["oX"], dtype=np.float64).max(axis=-1)
        )
        pmx = np.maximum(pmx, v.T.reshape(N))  # row = m*128+p

    # ---- host part: special 512-column block, exact in float64
    fq = feat_q.astype(np.float64)
    fk = feat_k.astype(np.float64)
    xx = (fq * fq).sum(1)
    kk_ = (fk * fk).sum(1)
    G = fq @ fk.T
    sqB = xx[:, None] + kk_[None, :] - 2.0 * G
    distB = np.sqrt(np.clip(sqB, 1e-12, None))
    maskB = t[:, None] == t[None, :]
    apB = np.max(distB - BIG * (~maskB), axis=1)
    anB = np.min(distB + BIG * maskB, axis=1)

    # zero-label region: ||z_j||^2 == 1; for t!=0 rows pmx = max_j p (hard
    # negative via min dist); for t==0 rows pmx = -min_j p (hard positive
    # via max dist)
    tz = t == 0
    an_z = np.where(
        tz, BIG, np.sqrt(np.clip(xx + 1.0 - 2.0 * pmx, 1e-12, None))
    )
    ap_z = np.where(
        tz, np.sqrt(np.clip(xx + 1.0 + 2.0 * pmx, 1e-12, None)), -BIG
    )

    dist_ap = np.maximum(apB, ap_z)
    dist_an = np.minimum(anB, an_z)
    return _loss(dist_ap, dist_an)


# revision 42
# speedup vs baseline: 1.1547x; 1.1547x over previous
"""MoCo hard-example-mining loss (topk_masking) on 8 Trainium2 NeuronCores.

Strategy (sharding_hint: shard queue along K):
  The reference computes dist = euclid(feat_q, queue_eff.T) [N=512, K=65536],
  then masked max (hard positive) / min (hard negative) per row, then a
  scalar soft-margin loss.  After the enqueue step, queue_eff columns are:
    - cols [0, 512):  feat_k.T with labels = targets   (the "special" block)
    - cols [512, 64K): original L2-normalized queue columns, labels = 0
  For the zero-label region the mask is row-constant and ||z_j||^2 == 1, so
  per row only an extreme of p_ij = <feat_q_i, z_j> over that region is
  needed — and only ONE side per row:
    - rows with target != 0: the region holds only negatives -> need max_j p
    - rows with target == 0: the region holds only positives -> need min_j p
  Sign-flipping the target==0 rows of feat_q on the host turns both cases
  into a single per-row MAX on device (min_j p = -max_j <-q, z>).  The
  512-column special block and the final scalar loss are computed exactly on
  the host in float64 (trivial cost).

  Device (per core, fp8e4 + DoubleRow): the 65024 zero-label columns are
  padded to 65536 with duplicate columns (harmless for max) and sharded 8192
  per core.  128 DoubleRow matmuls (256-deep contraction, 512-wide moving)
  fill [128, 1024] 2-bank PSUM pairs; one DVE max tensor_reduce per pair
  accumulates into a [128, 4, 8] slot tile; a final tiny reduce + 2 KB DMA
  returns [128, 4] row maxima (row index = m*128 + p).  Host reduces across
  cores.  fp8e4 noise on p (~0.05 abs on extremes ~100) is far inside the
  2e-2 loss tolerance.
"""

import sys
import types
import numpy as np
import ml_dtypes

N, DIM, K, B = 512, 512, 65536, 512
NCORES = 8
KZ = K - B            # zero-label columns
CPC = K // NCORES     # padded columns per core (8192)
NT = CPC // 512       # 512-wide column tiles per core (16)
BIG = 9999999.0

LAST_RESULTS = None   # BassKernelResults of the most recent device run
_NC_CACHE = {}


def _install_axon_hooks_shim():
    """antenv.axon_hooks is absent on this image; bass_utils imports it when
    NTFF tracing is requested.  Provide the tiny get/set module and register
    the ctypes-based NTFF hook so trace=True / BASS_TRACE=1 works."""
    try:
        import antenv  # noqa: F401
    except ImportError:
        return
    if "antenv.axon_hooks" in sys.modules:
        return
    mod = types.ModuleType("antenv.axon_hooks")
    mod._hook = None

    def set_axon_ntff_profile_hook(h):
        mod._hook = h

    def get_axon_ntff_profile_hook():
        return mod._hook

    mod.set_axon_ntff_profile_hook = set_axon_ntff_profile_hook
    mod.get_axon_ntff_profile_hook = get_axon_ntff_profile_hook
    sys.modules["antenv.axon_hooks"] = mod
    sys.modules["antenv"].axon_hooks = mod
    try:
        from trn_agent_boot.trn_boot import _ntff_profile_via_ctypes

        mod._hook = _ntff_profile_via_ctypes("/opt/axon/libaxon_pjrt.so")
    except Exception:
        pass


def _build_nc():
    """Build + compile the per-core Bass program (identical on all cores)."""
    import concourse.bacc as bacc
    import concourse.mybir as mybir
    from concourse.tile import TileContext

    f8 = mybir.dt.float8e4
    f32 = mybir.dt.float32
    DR = mybir.MatmulPerfMode.DoubleRow

    nc = bacc.Bacc("TRN2", debug=False, target_bir_lowering=False)
    bf16 = mybir.dt.bfloat16
    # qT8[p, kk, i] = feat_q'[i, kk*128+p]  (sign-flipped rows for target==0)
    qT = nc.dram_tensor("qT8", [128, 4, N], f8, kind="ExternalInput")
    # slab8[p, 4*n+kk, c] = z[kk*128+p, n*512+c]  (per-core column slab)
    slab = nc.dram_tensor("slab8", [128, 4 * NT, 512], f8, kind="ExternalInput")
    # o[p, m, path, c]: running-max accumulators per m block (path 0 =
    # direct DVE-from-PSUM, path 1 = ACT-copy + DVE bf16 TT, path 2 = solo
    # cast of the very last PSUM pair — written for m=3 only); the final
    # fold over (path, c) happens on the host.  Row index = m*128 + p.
    o = nc.dram_tensor("o", [128, 4, 3, 1024], bf16, kind="ExternalOutput")

    # Per-m evacuation path of each of the 8 column-pair PSUM tiles:
    #   F = DVE tensor_tensor max straight from PSUM (fp32 src, 1x, ~1.2us)
    #   D = ScalarE copy psum->bf16 (~1.15us) + DVE bf16 TT max (2x, ~0.7us)
    # 2F/6D balances DVE ~25us vs ACT ~27.5us, both under the ~29us matmul
    # stream.  tensor_reduce is NOT used in the hot path (only a 1x uop on
    # TRN2), and GpSimd tensor ops fail the PJRT-side walrus engine check.
    # For the last m the F pairs go last so the D accumulator (and its DMA)
    # retires before the stream ends.
    # Per-m path lists.  m0 ends on an F pair (DVE-only, retires early), so
    # the ScalarE copy queue drains before the last steps; m3's final pair is
    # a solo cast into its own accumulator (path X, shipped as two half-DMAs)
    # so only ~1.4us of DVE + a 128KB DMA trail the final matmul.
    PATHS_M = [
        ["F", "D", "D", "D", "F", "D", "D", "D"],
        ["F", "D", "D", "D", "F", "D", "D", "D"],
        ["F", "D", "D", "D", "F", "D", "D", "D"],
        ["D", "D", "D", "F", "D", "F", "F", "X"],
    ]
    # (m, pair) issue order: 4-way round-robin over the m blocks.  Each slab
    # tile pair is consumed over 4 consecutive steps (~3.6us), far behind the
    # ~1.5us/tile DMA arrival rate, there are no sharp m-boundaries (whose
    # evac-op clusters stalled PSUM slot reuse), and the last m's ScalarE
    # copies spread across the whole stream instead of bunching at the end.
    SCHED = [(m, p) for p in range(NT // 2) for m in range(4)]

    with TileContext(nc) as tc:
        with (
            tc.tile_pool(name="qpool", bufs=1) as qpool,
            tc.tile_pool(name="spool", bufs=16) as spool,
            tc.tile_pool(name="btpool", bufs=6) as btpool,
            tc.tile_pool(name="apool", bufs=4) as apool,
            tc.tile_pool(name="pspool", bufs=4, space="PSUM") as pspool,
        ):
            # HAM warmup: FD-512 matmuls bridge the whole preamble+DMA-latency
            # window (~3.4 us at the cold 1.2 GHz clock) so the PE activity
            # monitor unthrottles to 2.4 GHz right as the real stream starts.
            warm = qpool.tile([128, 512], f8, name="warm")
            nc.gpsimd.memset(warm, 0.0)
            # qt rides the second HWDGE ring (ScalarE) so its ~3us transfer
            # overlaps st0's on the Sync ring and every slab tile lands one
            # issue-slot earlier
            qt = qpool.tile([128, 4, N], f8, name="qt")
            nc.scalar.dma_start(out=qt, in_=qT.ap())
            # dummy ACTIVATE (after the qt issue) so walrus's lazily-inserted
            # ACT_TABLE_LOAD (~2.7 us) runs during the preamble window instead
            # of delaying the first real psum copy
            wact = qpool.tile([128, 8], bf16, name="wact")
            nc.scalar.copy(wact, warm[:, 0:8])
            wps = pspool.tile([128, 2, 512], f32, name="wps", tag="ps")
            for _ in range(9):
                nc.tensor.matmul(wps[0:16, 0, :], warm[:, 0:16], warm)

            # stage the whole 4 MB slab (resident: 16 x 2KB/partition)
            sts = []
            for n in range(NT):
                st = spool.tile([128, 4, 512], f8, name="st", tag="st")
                nc.sync.dma_start(out=st, in_=slab.ap()[:, 4 * n : 4 * n + 4, :])
                sts.append(st)

            paths_by_m = PATHS_M
            accs = {}  # m -> {path: [tile, has_data]}
            for m, pair in SCHED:
                paths = paths_by_m[m]
                if m not in accs:
                    accs[m] = {}
                    for pth in set(paths):
                        t = apool.tile(
                            [128, 1024], bf16, name="acc", tag="a" + pth
                        )
                        accs[m][pth] = [t, False]
                ps = pspool.tile([128, 2, 512], f32, name="ps", tag="ps")
                for half in range(2):
                    for kp in range(2):
                        nc.tensor.matmul(
                            ps[:, half, :],
                            qt[:, 2 * kp : 2 * kp + 2, m * 128 : (m + 1) * 128],
                            sts[pair * 2 + half][:, 2 * kp : 2 * kp + 2, :],
                            start=(kp == 0),
                            stop=(kp == 1),
                            perf_mode=DR,
                        )
                psf = ps.rearrange("p a b -> p (a b)")
                pth = paths[pair]
                at, has = accs[m][pth]
                if pth == "X":
                    # two half reduces (tensor_reduce is 1x but only 512 elems
                    # here; the first overlaps the final accumulation group)
                    # and a 1KB trailing DMA
                    nc.vector.tensor_reduce(
                        at[:, 0:1], ps[:, 0, :],
                        axis=mybir.AxisListType.X, op=mybir.AluOpType.max,
                    )
                    nc.vector.tensor_reduce(
                        at[:, 1:2], ps[:, 1, :],
                        axis=mybir.AxisListType.X, op=mybir.AluOpType.max,
                    )
                    nc.sync.dma_start(out=o.ap()[:, m, 2, 0:2], in_=at[:, 0:2])
                    continue
                elif pth == "F":
                    if not has:
                        nc.vector.tensor_copy(at, psf)
                    else:
                        nc.vector.tensor_tensor(at, at, psf, op=mybir.AluOpType.max)
                else:
                    bt = btpool.tile([128, 1024], bf16, name="bt", tag="bt")
                    nc.scalar.copy(bt, psf)
                    if not has:
                        nc.vector.tensor_copy(at, bt)
                    else:
                        nc.vector.tensor_tensor(at, at, bt, op=mybir.AluOpType.max)
                accs[m][pth][1] = True
                if paths[pair + 1 :].count(pth) == 0:
                    # last contribution for this accumulator: ship it
                    k = {"F": 0, "D": 1, "X": 2}[pth]
                    nc.sync.dma_start(out=o.ap()[:, m, k, :], in_=at)

    nc.compile()
    return nc


def _get_nc():
    if "nc" not in _NC_CACHE:
        _install_axon_hooks_shim()
        _NC_CACHE["nc"] = _build_nc()
    return _NC_CACHE["nc"]


def _host_reference(feat_q, feat_k, targets, queue, queue_label):
    """Exact numpy fallback (float64) — used only if input assumptions
    (zero labels / normalized columns outside the enqueue block) fail."""
    fq = feat_q.astype(np.float64)
    fk = feat_k.astype(np.float64)
    t = targets.astype(np.int64)
    q = queue.astype(np.float64).copy()
    ql = queue_label.astype(np.int64).copy()
    q[:, : fk.shape[0]] = fk.T
    ql[: fk.shape[0]] = t
    xx = (fq * fq).sum(1)[:, None]
    yy = (q * q).sum(0)[None, :]
    sq = xx + yy - 2.0 * (fq @ q)
    dist = np.sqrt(np.clip(sq, 1e-12, None))
    is_pos = t[:, None] == ql[None, :]
    dist_ap = np.max(dist - BIG * (~is_pos), axis=1)
    dist_an = np.min(dist + BIG * is_pos, axis=1)
    return _loss(dist_ap, dist_an)


def _loss(dist_ap, dist_an):
    diff = dist_an - dist_ap
    loss_soft = np.mean(np.logaddexp(0.0, -diff))
    if np.isinf(loss_soft):
        return np.float32(np.mean(np.maximum(dist_ap - dist_an + 0.3, 0.0)))
    return np.float32(loss_soft)


def kernel(feat_q, feat_k, targets, queue, queue_label):
    feat_q = np.asarray(feat_q, dtype=np.float32)
    feat_k = np.asarray(feat_k, dtype=np.float32)
    targets = np.asarray(targets)
    queue = np.asarray(queue, dtype=np.float32)
    queue_label = np.asarray(queue_label)

    t = targets.astype(np.int64)
    Z = queue[:, B:]  # zero-label region, untouched by the enqueue

    # Guards for the two structural assumptions this split relies on.
    ok = not np.any(queue_label != 0)
    if ok:
        sample = np.linspace(0, KZ - 1, 512, dtype=np.int64)
        yy_s = np.einsum("ij,ij->j", Z[:, sample], Z[:, sample], dtype=np.float64)
        ok = bool(np.max(np.abs(yy_s - 1.0)) < 1e-3)
    if not ok:
        return _host_reference(feat_q, feat_k, targets, queue, queue_label)

    # ---- device part: per-row max of feat_q' @ Z over the zero-label region
    fp8 = ml_dtypes.float8_e4m3
    sign = np.where(t == 0, -1.0, 1.0).astype(np.float32)
    fq8 = (feat_q * sign[:, None]).astype(fp8)          # [N, dim]
    qtd = np.ascontiguousarray(fq8.T.reshape(4, 128, N).transpose(1, 0, 2))
    Z8 = Z.astype(fp8)                                   # [dim, KZ]
    in_maps = []
    for c in range(NCORES):
        lo = c * CPC
        hi = min((c + 1) * CPC, KZ)
        sl = np.empty((DIM, CPC), dtype=fp8)
        sl[:, : hi - lo] = Z8[:, lo:hi]
        if hi - lo < CPC:  # pad the tail core with duplicate columns
            sl[:, hi - lo :] = Z8[:, : CPC - (hi - lo)]
        sld = np.ascontiguousarray(
            sl.reshape(4, 128, NT, 512).transpose(1, 2, 0, 3).reshape(128, 4 * NT, 512)
        )
        in_maps.append({"qT8": qtd, "slab8": sld})

    from concourse import bass_utils

    nc = _get_nc()
    res = bass_utils.run_bass_kernel_spmd(nc, in_maps, core_ids=list(range(NCORES)))
    global LAST_RESULTS
    LAST_RESULTS = res

    pmx = np.full(N, -np.inf)
    for c in range(NCORES):
        oc = np.asarray(res.results[c]["o"], dtype=np.float64)  # [128,4,3,1024]
        v = oc[:, :, 0:2, :].max(axis=(2, 3))  # [128, 4]; path 2 is m=3 only
        v[:, 3] = np.maximum(v[:, 3], oc[:, 3, 2, 0:2].max(axis=-1))
        pmx = np.maximum(pmx, v.T.reshape(N))  # row = m*128+p

    # ---- host part: special 512-column block, exact in float64
    fq = feat_q.astype(np.float64)
    fk = feat_k.astype(np.float64)
    xx = (fq * fq).sum(1)
    kk_ = (fk * fk).sum(1)
    G = fq @ fk.T
    sqB = xx[:, None] + kk_[None, :] - 2.0 * G
    distB = np.sqrt(np.clip(sqB, 1e-12, None))
    maskB = t[:, None] == t[None, :]
    apB = np.max(distB - BIG * (~maskB), axis=1)
    anB = np.min(distB + BIG * maskB, axis=1)

    # zero-label region: ||z_j||^2 == 1; for t!=0 rows pmx = max_j p (hard
    # negative via min dist); for t==0 rows pmx = -min_j p (hard positive
    # via max dist)
    tz = t == 0
    an_z = np.where(
        tz, BIG, np.sqrt(np.clip(xx + 1.0 - 2.0 * pmx, 1e-12, None))
    )
    ap_z = np.where(
        tz, np.sqrt(np.clip(xx + 1.0 + 2.0 * pmx, 1e-12, None)), -BIG
    )

    dist_ap = np.maximum(apB, ap_z)
    dist_an = np.minimum(anB, an_z)
    return _loss(dist_ap, dist_an)
